# revision 13
# baseline (speedup 1.0000x reference)
"""BlockDiagonalGRU Trainium2 kernel.

Math (per batch row b, per block n of 8, BLK=256):
  gates[b, n, :] = x[b, n*256:(n+1)*256] @ w_ih[n].T + h[b, ...] @ w_hh[n].T + b_ih[n] + b_hh[n]
  r = sigmoid(gates[..., 0:256]); u = sigmoid(gates[..., 256:512])
  c = tanh(r * gates[..., 512:768])
  h_new = (1-u)*h_blk + u*c

Sharding: data-parallel over batch across 8 cores (2048 rows each), weights
replicated (pre-transposed/cast on host).

All transposes are done on the host: per batch tile of 128 rows, the DRAM
tensor `xh` holds the feature-major layout [128 f_lo, s(2), 16 f_hi, 128 b]
in bf16 so matmul lhsT chunks are direct SBUF slices (no PE transposes).
Mixed precision on the PE (validated against the reference in fp64 numpy,
rel err ~1.5e-2 vs the 2e-2 gate):
  - r gate: fp8 e4m3 both paths (DoubleRow perf mode, 2x PE rate); its
    error is damped through tanh(r*g_c)
  - u gate: fp8 x-path, bf16 h-path
  - c gate: bf16 both paths
Inputs are duplicated in fp8 (scaled by ALPHA=1/4, fp8 weights by 4) so
PSUM lands at true scale. h is additionally loaded natural [b, f] bf16 for
the blend. Per core, software-pipelined over 16 batch tiles:
  - HWDGE load xh (1 MB) + xh8 (0.5 MB) two tiles ahead, h_nat (0.5 MB)
    one tile ahead (it is first needed a tile later, easing the startup
    bandwidth crunch); weights loaded block-major in first-use order
  - PE per block: r: 2 fp8 DR matmuls; u: 1 fp8 DR + 2 bf16; c: 4 bf16
  - ACT: one sigmoid over [r|u] PSUM -> SBUF bf16; DVE: r*g_c; ACT: tanh
  - blend in bf16 on DVE, pipelined one tile behind; last two tiles blend
    per-group to shorten the tail; HWDGE store bf16 (host casts to fp32).
"""

import numpy as np
import ml_dtypes

NUM_BLOCKS = 8
BLK = 256
D = 2048
B = 16384
N_CORES = 8
B_LOC = B // N_CORES  # 2048
P = 128
NBT = B_LOC // P  # 16 batch tiles per core
KC = 2  # k-chunks of 128 per block (256 feat)
G3 = 3 * BLK  # 768
BPG = 2  # blocks per PSUM group
NGRP = NUM_BLOCKS // BPG  # 4
NT = D // P  # 16 feat chunks
ALPHA = 0.25  # fp8 scale split: x*ALPHA, w8/ALPHA

_nc_cache = {}
_lut_cache = {}


def _build(has_bias, reps=1):
    import concourse.mybir as mybir
    import concourse.tile as tile
    from concourse import bacc
    from concourse.masks import make_identity

    f32 = mybir.dt.float32
    bf16 = mybir.dt.bfloat16
    fp8 = mybir.dt.float8e4
    DR = mybir.MatmulPerfMode.DoubleRow
    Sig = mybir.ActivationFunctionType.Sigmoid
    Tanh = mybir.ActivationFunctionType.Tanh

    nc = bacc.Bacc(None, target_bir_lowering=False)

    xh_d = nc.dram_tensor("xh", [NBT * P, 2 * NT * P], bf16, kind="ExternalInput")
    xh8_d = nc.dram_tensor("xh8", [NBT * P, 2 * NT * P], fp8, kind="ExternalInput")
    hn_d = nc.dram_tensor("hn", [B_LOC, D], bf16, kind="ExternalInput")
    # bf16 weights, block-major: [p, n, {u_s1, c_s0, c_s1}(3), kc, BLK]
    wt16_d = nc.dram_tensor("wt16", [P, NUM_BLOCKS * 3 * KC * BLK], bf16, kind="ExternalInput")
    # fp8 weights, block-major: [p, n, {r_s0, r_s1, u_s0}(3), kc, BLK]
    wt8_d = nc.dram_tensor("wt8", [P, NUM_BLOCKS * 3 * KC * BLK], fp8, kind="ExternalInput")
    if has_bias:
        bias_d = nc.dram_tensor("bias", [P, NUM_BLOCKS * G3], f32, kind="ExternalInput")
    out_d = nc.dram_tensor("out", [B_LOC, D], bf16, kind="ExternalOutput")
    warm_d = nc.dram_tensor("warm_scratch", [P, P], mybir.dt.bfloat16)

    with tile.TileContext(nc) as tc:
        with (
            tc.tile_pool(name="const", bufs=1) as cpool,
            tc.tile_pool(name="io", bufs=4) as io,
            tc.tile_pool(name="work", bufs=2) as work,
            tc.tile_pool(name="psru", bufs=3, space="PSUM") as psru_pool,
            tc.tile_pool(name="psc", bufs=2, space="PSUM") as psc_pool,
        ):
            ident = cpool.tile([P, P], bf16)
            make_identity(nc, ident)
            wt16 = cpool.tile([P, NUM_BLOCKS, 3, KC, BLK], bf16)
            wt8 = cpool.tile([P, NUM_BLOCKS, 3, KC, BLK], fp8)
            if has_bias:
                bias_sb = cpool.tile([P, NUM_BLOCKS, 3, BLK], f32)

            def load_wt_block(n):
                # one block's fp8 and bf16 weights; alternate HWDGE rings so
                # the chunks share the SDMA engines during the startup crunch
                cw = 3 * KC * BLK
                eng8 = nc.scalar if n % 2 == 0 else nc.sync
                eng16 = nc.sync if n % 2 == 0 else nc.scalar
                eng8.dma_start(wt8[:, n, :, :, :], wt8_d[:, n * cw : (n + 1) * cw])
                eng16.dma_start(wt16[:, n, :, :, :], wt16_d[:, n * cw : (n + 1) * cw])

            def load_tile(bt):
                row0 = bt * P
                xh_t = io.tile([P, 2 * NT * P], bf16, tag="xh", name="xh")
                nc.gpsimd.dma_start(xh_t[:], xh_d[row0 : row0 + P, :])
                xh8_t = io.tile([P, 2 * NT * P], fp8, tag="xh8", name="xh8")
                nc.scalar.dma_start(xh8_t[:], xh8_d[row0 : row0 + P, :])
                return xh_t, xh8_t

            def load_hn(bt):
                row0 = bt * P
                hn_t = io.tile([P, D], bf16, tag="hn", name="hn")
                nc.sync.dma_start(hn_t[:], hn_d[row0 : row0 + P, :])
                return hn_t

            def gates_group(bt, grp, xh_t, xh8_t, u_buf, c_buf, fine=None):
                ps_ru = psru_pool.tile([P, BPG * 2 * BLK], f32, tag="psru", name="psru")
                ps_c = psc_pool.tile([P, BPG * BLK], f32, tag="psc", name="psc")
                for nn in range(BPG):
                    n = grp * BPG + nn
                    ru0 = nn * 2 * BLK

                    def lhsT8(s):
                        c0 = (s * NT + n * KC) * P
                        return xh8_t[:, c0 : c0 + 2 * P].rearrange(
                            "p (k b) -> p k b", k=2
                        )

                    # r gate: fp8 DoubleRow, both paths
                    for s in (0, 1):
                        nc.tensor.matmul(
                            ps_ru[:, ru0 : ru0 + BLK],
                            lhsT8(s),
                            wt8[:, n, s, :, :],
                            start=(s == 0),
                            stop=(s == 1),
                            perf_mode=DR,
                        )
                    # u gate: fp8 DR x-path, then bf16 h-path
                    nc.tensor.matmul(
                        ps_ru[:, ru0 + BLK : ru0 + 2 * BLK],
                        lhsT8(0),
                        wt8[:, n, 2, :, :],
                        start=True,
                        stop=False,
                        perf_mode=DR,
                    )
                    for kc in range(KC):
                        lhsT = xh_t[:, (NT + n * KC + kc) * P :][:, :P]
                        nc.tensor.matmul(
                            ps_ru[:, ru0 + BLK : ru0 + 2 * BLK],
                            lhsT,
                            wt16[:, n, 0, kc, :],
                            start=False,
                            stop=(kc == KC - 1),
                        )
                    # c gate: bf16 both paths
                    for s in (0, 1):
                        for kc in range(KC):
                            lhsT = xh_t[:, (s * NT + n * KC + kc) * P :][:, :P]
                            nc.tensor.matmul(
                                ps_c[:, nn * BLK : (nn + 1) * BLK],
                                lhsT,
                                wt16[:, n, 1 + s, kc, :],
                                start=(s == 0 and kc == 0),
                                stop=(s == 1 and kc == KC - 1),
                            )
                if has_bias:
                    for nn in range(BPG):
                        n = grp * BPG + nn
                        for g in range(2):
                            sl = slice(nn * 2 * BLK + g * BLK, nn * 2 * BLK + (g + 1) * BLK)
                            nc.vector.tensor_add(ps_ru[:, sl], ps_ru[:, sl], bias_sb[:, n, g, :])
                        nc.vector.tensor_add(
                            ps_c[:, nn * BLK : (nn + 1) * BLK],
                            ps_c[:, nn * BLK : (nn + 1) * BLK],
                            bias_sb[:, n, 2, :],
                        )
                if fine is not None:
                    # last-tile drain: per-block activations + blend + store so
                    # the post-matmul tail chain is as short as possible
                    bt_, h_nat = fine
                    row0 = bt_ * P
                    for nn in range(BPG):
                        n = grp * BPG + nn
                        c0 = n * BLK
                        nc.scalar.activation(
                            u_buf[:, n * 2 * BLK : (n + 1) * 2 * BLK],
                            ps_ru[:, nn * 2 * BLK : (nn + 1) * 2 * BLK],
                            Sig,
                        )
                        rcb = work.tile([P, BLK], bf16, tag="rcb", name="rcb", bufs=2)
                        nc.vector.tensor_mul(
                            rcb[:],
                            u_buf[:, n * 2 * BLK : n * 2 * BLK + BLK],
                            ps_c[:, nn * BLK : (nn + 1) * BLK],
                        )
                        nc.scalar.activation(c_buf[:, c0 : c0 + BLK], rcb[:], Tanh)
                        d_b = work.tile([P, BLK], bf16, tag="d_b", name="d_b", bufs=2)
                        e_b = work.tile([P, BLK], bf16, tag="e_b", name="e_b", bufs=2)
                        nc.vector.tensor_sub(
                            d_b[:], c_buf[:, c0 : c0 + BLK], h_nat[:, c0 : c0 + BLK]
                        )
                        nc.vector.tensor_mul(
                            e_b[:], u_buf[:, n * 2 * BLK + BLK : (n + 1) * 2 * BLK], d_b[:]
                        )
                        hnew = work.tile([P, BLK], bf16, tag="hnew_b", name="hnew_b", bufs=2)
                        nc.vector.tensor_add(hnew[:], h_nat[:, c0 : c0 + BLK], e_b[:])
                        nc.sync.dma_start(out_d[row0 : row0 + P, c0 : c0 + BLK], hnew[:])
                    return
                col0 = grp * BPG * BLK
                col1 = (grp + 1) * BPG * BLK
                # one sigmoid over the whole [r|u] PSUM tile -> interleaved ru_buf
                ruc0 = grp * BPG * 2 * BLK
                ruc1 = (grp + 1) * BPG * 2 * BLK
                nc.scalar.activation(u_buf[:, ruc0:ruc1], ps_ru[:], Sig)
                r3 = u_buf[:, ruc0:ruc1].rearrange("p (a g b) -> p a g b", a=BPG, g=2)[
                    :, :, 0, :
                ]
                rc = work.tile([P, BPG * BLK], bf16, tag="rc", name="rc", bufs=3)
                nc.vector.tensor_mul(
                    rc[:].rearrange("p (a b) -> p a b", a=BPG),
                    r3,
                    ps_c[:].rearrange("p (a b) -> p a b", a=BPG),
                )
                nc.scalar.activation(c_buf[:, col0:col1], rc[:], Tanh)

            def u_view(u_buf, col0, col1):
                # u slices of the interleaved [r|u] buffer covering hidden
                # columns [col0, col1)
                nblk = (col1 - col0) // BLK
                return u_buf[:, 2 * col0 : 2 * col1].rearrange(
                    "p (a g b) -> p a g b", a=nblk, g=2
                )[:, :, 1, :]

            def blend_full(bt, h_nat, u_buf, c_buf):
                row0 = bt * P
                d_t = work.tile([P, D], bf16, tag="d_t", name="d_t")
                e_t = work.tile([P, D], bf16, tag="e_t", name="e_t")
                nc.vector.tensor_sub(d_t[:], c_buf[:], h_nat[:])
                nc.vector.tensor_mul(
                    e_t[:].rearrange("p (a b) -> p a b", b=BLK),
                    u_view(u_buf, 0, D),
                    d_t[:].rearrange("p (a b) -> p a b", b=BLK),
                )
                hnew = work.tile([P, D], bf16, tag="hnew", name="hnew")
                nc.vector.tensor_add(hnew[:], h_nat[:], e_t[:])
                nc.sync.dma_start(out_d[row0 : row0 + P, :], hnew[:])

            def blend_grp(bt, grp, h_nat, u_buf, c_buf):
                row0 = bt * P
                col0 = grp * BPG * BLK
                col1 = (grp + 1) * BPG * BLK
                d_t = work.tile([P, BPG * BLK], bf16, tag="d_g", name="d_g")
                e_t = work.tile([P, BPG * BLK], bf16, tag="e_g", name="e_g")
                nc.vector.tensor_sub(d_t[:], c_buf[:, col0:col1], h_nat[:, col0:col1])
                nc.vector.tensor_mul(
                    e_t[:].rearrange("p (a b) -> p a b", b=BLK),
                    u_view(u_buf, col0, col1),
                    d_t[:].rearrange("p (a b) -> p a b", b=BLK),
                )
                hnew = work.tile([P, BPG * BLK], bf16, tag="hnew_g", name="hnew_g")
                nc.vector.tensor_add(hnew[:], h_nat[:, col0:col1], e_t[:])
                nc.sync.dma_start(out_d[row0 : row0 + P, col0:col1], hnew[:])

            def warmup():
                # dummy matmuls while the first loads are in flight: keeps the
                # PE HAM activity monitor busy so real matmuls start at 2.4GHz
                ps = psc_pool.tile([P, BPG * BLK], f32, tag="psc", name="psc_warm")
                NWU = 100
                for i in range(NWU):
                    nc.tensor.matmul(
                        ps[:, 0:P],
                        ident[:],
                        ident[:],
                        start=(i == 0),
                        stop=(i == NWU - 1),
                    )
                sc = work.tile([P, P], bf16, tag="warm_sb", name="warm_sb", bufs=1)
                nc.vector.tensor_copy(sc[:], ps[:, 0:P])
                nc.scalar.dma_start(warm_d[:, :], sc[:])

            def body(_iv=None):
                warmup()
                nats = {0: load_tile(0)}
                # weights block-major in first-use order, interleaved with the
                # first tile loads
                load_wt_block(0)
                load_wt_block(1)
                load_wt_block(2)
                nats[1] = load_tile(1)
                load_wt_block(3)
                load_wt_block(4)
                load_wt_block(5)
                hns = {0: load_hn(0)}
                load_wt_block(6)
                load_wt_block(7)
                if has_bias:
                    nc.sync.dma_start(bias_sb[:, :, :, :], bias_d[:, :])
                pending = None
                for bt in range(NBT):
                    xh_t, xh8_t = nats.pop(bt)
                    h_nat = hns.pop(bt)

                    # interleaved [r|u] sigmoid outputs: [128, n(8) x {r,u} x 256]
                    u_buf = work.tile([P, 2 * D], bf16, tag="u_buf", name="u_buf", bufs=3)
                    c_buf = work.tile([P, D], bf16, tag="c_buf", name="c_buf", bufs=3)

                    for grp in range(NGRP):
                        if grp == 1 and bt + 1 < NBT:
                            # h natural is first consumed a tile after its
                            # gates, so one-tile prefetch is enough
                            hns[bt + 1] = load_hn(bt + 1)
                        if grp == 3 and bt + 2 < NBT:
                            # prefetch two tiles ahead, late in the loop so the
                            # startup weight DMAs win the early SDMA bandwidth
                            nats[bt + 2] = load_tile(bt + 2)
                        gates_group(bt, grp, xh_t, xh8_t, u_buf, c_buf)
                        if bt >= NBT - 2:
                            blend_grp(bt, grp, h_nat, u_buf, c_buf)
                    if pending is not None:
                        blend_full(*pending)
                        pending = None
                    if bt < NBT - 2:
                        pending = (bt, h_nat, u_buf, c_buf)

            if reps == 1:
                body()
            else:
                with tc.For_i(0, reps, 1) as iv:
                    body(iv)

    nc.compile()
    return nc


def _get_nc(has_bias, reps=1):
    key = (has_bias, reps)
    if key not in _nc_cache:
        _nc_cache[key] = _build(has_bias, reps)
    return _nc_cache[key]


def _bf16_to_fp8_lut():
    # LUT over all bf16 bit patterns: fp8e4m3(ALPHA * value)
    if "lut" not in _lut_cache:
        allbits = np.arange(65536, dtype=np.uint16)
        vals = allbits.view(ml_dtypes.bfloat16).astype(np.float32)
        _lut_cache["lut"] = (vals * ALPHA).astype(ml_dtypes.float8_e4m3)
    return _lut_cache["lut"]


def _prep_weights(w_ih, w_hh):
    # wt8 [p, n, {r_s0, r_s1, u_s0}, kc, :] (scaled 1/ALPHA, fp8)
    # wt16[p, n, {u_s1, c_s0, c_s1}, kc, :] (bf16)
    def tr(w):
        # [n, g3, kc, p] -> [p, n, kc, g3]
        return w.reshape(NUM_BLOCKS, G3, KC, P).transpose(3, 0, 2, 1)

    wb = np.stack([tr(w_ih), tr(w_hh)], axis=2)  # [p, n, s, kc, g3]
    r_ = wb[..., 0:BLK]
    u_ = wb[..., BLK : 2 * BLK]
    c_ = wb[..., 2 * BLK :]
    w8 = np.stack([r_[:, :, 0], r_[:, :, 1], u_[:, :, 0]], axis=2) * (1.0 / ALPHA)
    w16 = np.stack([u_[:, :, 1], c_[:, :, 0], c_[:, :, 1]], axis=2)
    wt8 = np.ascontiguousarray(w8.reshape(P, -1).astype(ml_dtypes.float8_e4m3))
    wt16 = np.ascontiguousarray(w16.reshape(P, -1).astype(ml_dtypes.bfloat16))
    return wt16, wt8


def _prep_inputs(x, h, w_ih, w_hh, bsum):
    """Build per-core input maps (host-side cast + transpose)."""
    bf16 = ml_dtypes.bfloat16
    has_bias = bool(np.any(bsum))
    wt16, wt8 = _prep_weights(w_ih, w_hh)

    xb = x.astype(bf16).view(np.uint16)
    hb = h.astype(bf16).view(np.uint16)
    # xh[core, bt*128+f, s*2048 + t*128 + b] = (x if s==0 else h)[.., bt*128+b, t*128+f]
    xc = xb.reshape(N_CORES, NBT, P, NT, P)  # [c, bt, b, t, f]
    hc = hb.reshape(N_CORES, NBT, P, NT, P)
    xh = np.stack([xc, hc], axis=0)  # [s, c, bt, b, t, f]
    xh = np.ascontiguousarray(xh.transpose(1, 2, 5, 0, 4, 3))  # [c, bt, f, s, t, b]
    xh = xh.reshape(N_CORES, NBT * P, 2 * NT * P)
    xh8 = _bf16_to_fp8_lut()[xh]
    xh = xh.view(bf16)
    hn = hb.reshape(N_CORES, B_LOC, D).view(bf16)

    in_maps = []
    for c in range(N_CORES):
        m = {
            "xh": np.ascontiguousarray(xh[c]),
            "xh8": np.ascontiguousarray(xh8[c]),
            "hn": np.ascontiguousarray(hn[c]),
            "wt16": wt16,
            "wt8": wt8,
        }
        if has_bias:
            brep = np.broadcast_to(
                bsum.reshape(1, NUM_BLOCKS * G3), (P, NUM_BLOCKS * G3)
            ).astype(np.float32)
            m["bias"] = np.ascontiguousarray(brep)
        in_maps.append(m)
    return has_bias, in_maps


def kernel(x, h, w_ih, w_hh, b_ih, b_hh, _reps=1, _nc=None):
    from concourse.bass_utils import run_bass_kernel_spmd

    x = np.asarray(x, dtype=np.float32)
    h = np.asarray(h, dtype=np.float32)
    w_ih = np.asarray(w_ih, dtype=np.float32)
    w_hh = np.asarray(w_hh, dtype=np.float32)
    bsum = np.asarray(b_ih, dtype=np.float32) + np.asarray(b_hh, dtype=np.float32)

    has_bias, in_maps = _prep_inputs(x, h, w_ih, w_hh, bsum)
    nc = _nc if _nc is not None else _get_nc(has_bias, _reps)

    res = run_bass_kernel_spmd(nc, in_maps, core_ids=list(range(N_CORES)))
    out = np.concatenate([res.results[c]["out"] for c in range(N_CORES)], axis=0)
    return np.ascontiguousarray(out.astype(np.float32))


# revision 15
# speedup vs baseline: 1.0379x; 1.0379x over previous
"""BlockDiagonalGRU Trainium2 kernel.

Math (per batch row b, per block n of 8, BLK=256):
  gates[b, n, :] = x[b, n*256:(n+1)*256] @ w_ih[n].T + h[b, ...] @ w_hh[n].T + b_ih[n] + b_hh[n]
  r = sigmoid(gates[..., 0:256]); u = sigmoid(gates[..., 256:512])
  c = tanh(r * gates[..., 512:768])
  h_new = (1-u)*h_blk + u*c

Sharding: data-parallel over batch across 8 cores (2048 rows each), weights
replicated (pre-transposed/cast on host).

All transposes are done on the host: per batch tile of 128 rows, the DRAM
tensor `xh` holds the feature-major layout [128 f_lo, s(2), 16 f_hi, 128 b]
in bf16 so matmul lhsT chunks are direct SBUF slices (no PE transposes).
Mixed precision on the PE (validated against the reference in fp64 numpy,
rel err ~1.5e-2 vs the 2e-2 gate):
  - r gate: fp8 e4m3 both paths (DoubleRow perf mode, 2x PE rate); its
    error is damped through tanh(r*g_c)
  - u gate: fp8 x-path, bf16 h-path
  - c gate: bf16 both paths
Inputs are duplicated in fp8 (scaled by ALPHA=1/4, fp8 weights by 4) so
PSUM lands at true scale. h is additionally loaded natural [b, f] bf16 for
the blend. Per core, software-pipelined over 16 batch tiles:
  - HWDGE load xh (1 MB) + xh8 (0.5 MB) two tiles ahead, h_nat (0.5 MB)
    one tile ahead (it is first needed a tile later, easing the startup
    bandwidth crunch); weights loaded block-major in first-use order
  - PE per block: r: 2 fp8 DR matmuls; u: 1 fp8 DR + 2 bf16; c: 4 bf16
  - ACT: one sigmoid over [r|u] PSUM -> SBUF bf16; DVE: r*g_c; ACT: tanh
  - blend in bf16 on DVE, pipelined one tile behind; last two tiles blend
    per-group to shorten the tail; HWDGE store bf16 (host casts to fp32).
"""

import numpy as np
import ml_dtypes

NUM_BLOCKS = 8
BLK = 256
D = 2048
B = 16384
N_CORES = 8
B_LOC = B // N_CORES  # 2048
P = 128
NBT = B_LOC // P  # 16 batch tiles per core
KC = 2  # k-chunks of 128 per block (256 feat)
G3 = 3 * BLK  # 768
BPG = 2  # blocks per PSUM group
NGRP = NUM_BLOCKS // BPG  # 4
NT = D // P  # 16 feat chunks
ALPHA = 0.25  # fp8 scale split: x*ALPHA, w8/ALPHA

_nc_cache = {}
_lut_cache = {}


def _build(has_bias, reps=1):
    import concourse.mybir as mybir
    import concourse.tile as tile
    from concourse import bacc
    from concourse.masks import make_identity

    f32 = mybir.dt.float32
    bf16 = mybir.dt.bfloat16
    fp8 = mybir.dt.float8e4
    DR = mybir.MatmulPerfMode.DoubleRow
    Sig = mybir.ActivationFunctionType.Sigmoid
    Tanh = mybir.ActivationFunctionType.Tanh

    nc = bacc.Bacc(None, target_bir_lowering=False)

    xh_d = nc.dram_tensor("xh", [NBT * P, 2 * NT * P], bf16, kind="ExternalInput")
    xh8_d = nc.dram_tensor("xh8", [NBT * P, 2 * NT * P], fp8, kind="ExternalInput")
    hn_d = nc.dram_tensor("hn", [B_LOC, D], bf16, kind="ExternalInput")
    # bf16 weights, block-major: [p, n, {u_s1, c_s0, c_s1}(3), kc, BLK]
    wt16_d = nc.dram_tensor("wt16", [P, NUM_BLOCKS * 3 * KC * BLK], bf16, kind="ExternalInput")
    # fp8 weights, block-major: [p, n, {r_s0, r_s1, u_s0}(3), kc, BLK]
    wt8_d = nc.dram_tensor("wt8", [P, NUM_BLOCKS * 3 * KC * BLK], fp8, kind="ExternalInput")
    if has_bias:
        bias_d = nc.dram_tensor("bias", [P, NUM_BLOCKS * G3], f32, kind="ExternalInput")
    out_d = nc.dram_tensor("out", [B_LOC, D], bf16, kind="ExternalOutput")
    warm_d = nc.dram_tensor("warm_scratch", [P, P], mybir.dt.bfloat16)

    with tile.TileContext(nc) as tc:
        with (
            tc.tile_pool(name="const", bufs=1) as cpool,
            tc.tile_pool(name="io", bufs=4) as io,
            tc.tile_pool(name="work", bufs=2) as work,
            tc.tile_pool(name="psru", bufs=3, space="PSUM") as psru_pool,
            tc.tile_pool(name="psc", bufs=2, space="PSUM") as psc_pool,
        ):
            ident = cpool.tile([P, P], bf16)
            make_identity(nc, ident)
            wt16 = cpool.tile([P, NUM_BLOCKS, 3, KC, BLK], bf16)
            wt8 = cpool.tile([P, NUM_BLOCKS, 3, KC, BLK], fp8)
            if has_bias:
                bias_sb = cpool.tile([P, NUM_BLOCKS, 3, BLK], f32)

            def load_wt_block(n):
                # one block's fp8 and bf16 weights; alternate HWDGE rings so
                # the chunks share the SDMA engines during the startup crunch
                cw = 3 * KC * BLK
                eng8 = nc.scalar if n % 2 == 0 else nc.sync
                eng16 = nc.sync if n % 2 == 0 else nc.scalar
                eng8.dma_start(wt8[:, n, :, :, :], wt8_d[:, n * cw : (n + 1) * cw])
                eng16.dma_start(wt16[:, n, :, :, :], wt16_d[:, n * cw : (n + 1) * cw])

            def load_tile(bt):
                row0 = bt * P
                xh_t = io.tile([P, 2 * NT * P], bf16, tag="xh", name="xh")
                nc.gpsimd.dma_start(xh_t[:], xh_d[row0 : row0 + P, :])
                xh8_t = io.tile([P, 2 * NT * P], fp8, tag="xh8", name="xh8")
                nc.gpsimd.dma_start(xh8_t[:], xh8_d[row0 : row0 + P, :])
                return xh_t, xh8_t

            def load_hn(bt):
                row0 = bt * P
                hn_t = io.tile([P, D], bf16, tag="hn", name="hn")
                nc.gpsimd.dma_start(hn_t[:], hn_d[row0 : row0 + P, :])
                return hn_t

            def gates_group(bt, grp, xh_t, xh8_t, u_buf, c_buf, fine=None):
                ps_ru = psru_pool.tile([P, BPG * 2 * BLK], f32, tag="psru", name="psru")
                ps_c = psc_pool.tile([P, BPG * BLK], f32, tag="psc", name="psc")
                for nn in range(BPG):
                    n = grp * BPG + nn
                    ru0 = nn * 2 * BLK

                    def lhsT8(s):
                        c0 = (s * NT + n * KC) * P
                        return xh8_t[:, c0 : c0 + 2 * P].rearrange(
                            "p (k b) -> p k b", k=2
                        )

                    # r gate: fp8 DoubleRow, both paths
                    for s in (0, 1):
                        nc.tensor.matmul(
                            ps_ru[:, ru0 : ru0 + BLK],
                            lhsT8(s),
                            wt8[:, n, s, :, :],
                            start=(s == 0),
                            stop=(s == 1),
                            perf_mode=DR,
                        )
                    # u gate: fp8 DR x-path, then bf16 h-path
                    nc.tensor.matmul(
                        ps_ru[:, ru0 + BLK : ru0 + 2 * BLK],
                        lhsT8(0),
                        wt8[:, n, 2, :, :],
                        start=True,
                        stop=False,
                        perf_mode=DR,
                    )
                    for kc in range(KC):
                        lhsT = xh_t[:, (NT + n * KC + kc) * P :][:, :P]
                        nc.tensor.matmul(
                            ps_ru[:, ru0 + BLK : ru0 + 2 * BLK],
                            lhsT,
                            wt16[:, n, 0, kc, :],
                            start=False,
                            stop=(kc == KC - 1),
                        )
                    # c gate: bf16 both paths
                    for s in (0, 1):
                        for kc in range(KC):
                            lhsT = xh_t[:, (s * NT + n * KC + kc) * P :][:, :P]
                            nc.tensor.matmul(
                                ps_c[:, nn * BLK : (nn + 1) * BLK],
                                lhsT,
                                wt16[:, n, 1 + s, kc, :],
                                start=(s == 0 and kc == 0),
                                stop=(s == 1 and kc == KC - 1),
                            )
                if has_bias:
                    for nn in range(BPG):
                        n = grp * BPG + nn
                        for g in range(2):
                            sl = slice(nn * 2 * BLK + g * BLK, nn * 2 * BLK + (g + 1) * BLK)
                            nc.vector.tensor_add(ps_ru[:, sl], ps_ru[:, sl], bias_sb[:, n, g, :])
                        nc.vector.tensor_add(
                            ps_c[:, nn * BLK : (nn + 1) * BLK],
                            ps_c[:, nn * BLK : (nn + 1) * BLK],
                            bias_sb[:, n, 2, :],
                        )
                if fine is not None:
                    # last-tile drain: per-block activations + blend + store so
                    # the post-matmul tail chain is as short as possible
                    bt_, h_nat = fine
                    row0 = bt_ * P
                    for nn in range(BPG):
                        n = grp * BPG + nn
                        c0 = n * BLK
                        nc.scalar.activation(
                            u_buf[:, n * 2 * BLK : (n + 1) * 2 * BLK],
                            ps_ru[:, nn * 2 * BLK : (nn + 1) * 2 * BLK],
                            Sig,
                        )
                        rcb = work.tile([P, BLK], bf16, tag="rcb", name="rcb", bufs=2)
                        nc.vector.tensor_mul(
                            rcb[:],
                            u_buf[:, n * 2 * BLK : n * 2 * BLK + BLK],
                            ps_c[:, nn * BLK : (nn + 1) * BLK],
                        )
                        nc.scalar.activation(c_buf[:, c0 : c0 + BLK], rcb[:], Tanh)
                        d_b = work.tile([P, BLK], bf16, tag="d_b", name="d_b", bufs=2)
                        e_b = work.tile([P, BLK], bf16, tag="e_b", name="e_b", bufs=2)
                        nc.vector.tensor_sub(
                            d_b[:], c_buf[:, c0 : c0 + BLK], h_nat[:, c0 : c0 + BLK]
                        )
                        nc.vector.tensor_mul(
                            e_b[:], u_buf[:, n * 2 * BLK + BLK : (n + 1) * 2 * BLK], d_b[:]
                        )
                        hnew = work.tile([P, BLK], bf16, tag="hnew_b", name="hnew_b", bufs=2)
                        nc.vector.tensor_add(hnew[:], h_nat[:, c0 : c0 + BLK], e_b[:])
                        nc.sync.dma_start(out_d[row0 : row0 + P, c0 : c0 + BLK], hnew[:])
                    return
                col0 = grp * BPG * BLK
                col1 = (grp + 1) * BPG * BLK
                # one sigmoid over the whole [r|u] PSUM tile -> interleaved ru_buf
                ruc0 = grp * BPG * 2 * BLK
                ruc1 = (grp + 1) * BPG * 2 * BLK
                nc.scalar.activation(u_buf[:, ruc0:ruc1], ps_ru[:], Sig)
                r3 = u_buf[:, ruc0:ruc1].rearrange("p (a g b) -> p a g b", a=BPG, g=2)[
                    :, :, 0, :
                ]
                rc = work.tile([P, BPG * BLK], bf16, tag="rc", name="rc", bufs=3)
                nc.vector.tensor_mul(
                    rc[:].rearrange("p (a b) -> p a b", a=BPG),
                    r3,
                    ps_c[:].rearrange("p (a b) -> p a b", a=BPG),
                )
                nc.scalar.activation(c_buf[:, col0:col1], rc[:], Tanh)

            def u_view(u_buf, col0, col1):
                # u slices of the interleaved [r|u] buffer covering hidden
                # columns [col0, col1)
                nblk = (col1 - col0) // BLK
                return u_buf[:, 2 * col0 : 2 * col1].rearrange(
                    "p (a g b) -> p a g b", a=nblk, g=2
                )[:, :, 1, :]

            def blend_full(bt, h_nat, u_buf, c_buf):
                row0 = bt * P
                d_t = work.tile([P, D], bf16, tag="d_t", name="d_t")
                e_t = work.tile([P, D], bf16, tag="e_t", name="e_t")
                nc.vector.tensor_sub(d_t[:], c_buf[:], h_nat[:])
                nc.vector.tensor_mul(
                    e_t[:].rearrange("p (a b) -> p a b", b=BLK),
                    u_view(u_buf, 0, D),
                    d_t[:].rearrange("p (a b) -> p a b", b=BLK),
                )
                hnew = work.tile([P, D], bf16, tag="hnew", name="hnew")
                nc.vector.tensor_add(hnew[:], h_nat[:], e_t[:])
                nc.sync.dma_start(out_d[row0 : row0 + P, :], hnew[:])

            def blend_grp(bt, grp, h_nat, u_buf, c_buf):
                row0 = bt * P
                col0 = grp * BPG * BLK
                col1 = (grp + 1) * BPG * BLK
                d_t = work.tile([P, BPG * BLK], bf16, tag="d_g", name="d_g")
                e_t = work.tile([P, BPG * BLK], bf16, tag="e_g", name="e_g")
                nc.vector.tensor_sub(d_t[:], c_buf[:, col0:col1], h_nat[:, col0:col1])
                nc.vector.tensor_mul(
                    e_t[:].rearrange("p (a b) -> p a b", b=BLK),
                    u_view(u_buf, col0, col1),
                    d_t[:].rearrange("p (a b) -> p a b", b=BLK),
                )
                hnew = work.tile([P, BPG * BLK], bf16, tag="hnew_g", name="hnew_g")
                nc.vector.tensor_add(hnew[:], h_nat[:, col0:col1], e_t[:])
                nc.sync.dma_start(out_d[row0 : row0 + P, col0:col1], hnew[:])

            def warmup():
                # dummy matmuls while the first loads are in flight: keeps the
                # PE HAM activity monitor busy so real matmuls start at 2.4GHz
                ps = psc_pool.tile([P, BPG * BLK], f32, tag="psc", name="psc_warm")
                NWU = 100
                for i in range(NWU):
                    nc.tensor.matmul(
                        ps[:, 0:P],
                        ident[:],
                        ident[:],
                        start=(i == 0),
                        stop=(i == NWU - 1),
                    )
                sc = work.tile([P, P], bf16, tag="warm_sb", name="warm_sb", bufs=1)
                nc.vector.tensor_copy(sc[:], ps[:, 0:P])
                nc.scalar.dma_start(warm_d[:, :], sc[:])

            def body(_iv=None):
                warmup()
                nats = {0: load_tile(0)}
                # weights block-major in first-use order, interleaved with the
                # first tile loads
                load_wt_block(0)
                load_wt_block(1)
                load_wt_block(2)
                nats[1] = load_tile(1)
                load_wt_block(3)
                load_wt_block(4)
                load_wt_block(5)
                hns = {0: load_hn(0)}
                load_wt_block(6)
                load_wt_block(7)
                if has_bias:
                    nc.sync.dma_start(bias_sb[:, :, :, :], bias_d[:, :])
                pending = None
                for bt in range(NBT):
                    xh_t, xh8_t = nats.pop(bt)
                    h_nat = hns.pop(bt)

                    # interleaved [r|u] sigmoid outputs: [128, n(8) x {r,u} x 256]
                    u_buf = work.tile([P, 2 * D], bf16, tag="u_buf", name="u_buf", bufs=3)
                    c_buf = work.tile([P, D], bf16, tag="c_buf", name="c_buf", bufs=3)

                    for grp in range(NGRP):
                        if grp == 1 and bt + 1 < NBT:
                            # h natural is first consumed a tile after its
                            # gates, so one-tile prefetch is enough
                            hns[bt + 1] = load_hn(bt + 1)
                        if grp == 3 and bt + 2 < NBT:
                            # prefetch two tiles ahead, late in the loop so the
                            # startup weight DMAs win the early SDMA bandwidth
                            nats[bt + 2] = load_tile(bt + 2)
                        gates_group(bt, grp, xh_t, xh8_t, u_buf, c_buf)
                        if bt >= NBT - 2:
                            blend_grp(bt, grp, h_nat, u_buf, c_buf)
                    if pending is not None:
                        blend_full(*pending)
                        pending = None
                    if bt < NBT - 2:
                        pending = (bt, h_nat, u_buf, c_buf)

            if reps == 1:
                body()
            else:
                with tc.For_i(0, reps, 1) as iv:
                    body(iv)

    nc.compile()
    return nc


def _get_nc(has_bias, reps=1):
    key = (has_bias, reps)
    if key not in _nc_cache:
        _nc_cache[key] = _build(has_bias, reps)
    return _nc_cache[key]


def _bf16_to_fp8_lut():
    # LUT over all bf16 bit patterns: fp8e4m3(ALPHA * value)
    if "lut" not in _lut_cache:
        allbits = np.arange(65536, dtype=np.uint16)
        vals = allbits.view(ml_dtypes.bfloat16).astype(np.float32)
        _lut_cache["lut"] = (vals * ALPHA).astype(ml_dtypes.float8_e4m3)
    return _lut_cache["lut"]


def _prep_weights(w_ih, w_hh):
    # wt8 [p, n, {r_s0, r_s1, u_s0}, kc, :] (scaled 1/ALPHA, fp8)
    # wt16[p, n, {u_s1, c_s0, c_s1}, kc, :] (bf16)
    def tr(w):
        # [n, g3, kc, p] -> [p, n, kc, g3]
        return w.reshape(NUM_BLOCKS, G3, KC, P).transpose(3, 0, 2, 1)

    wb = np.stack([tr(w_ih), tr(w_hh)], axis=2)  # [p, n, s, kc, g3]
    r_ = wb[..., 0:BLK]
    u_ = wb[..., BLK : 2 * BLK]
    c_ = wb[..., 2 * BLK :]
    w8 = np.stack([r_[:, :, 0], r_[:, :, 1], u_[:, :, 0]], axis=2) * (1.0 / ALPHA)
    w16 = np.stack([u_[:, :, 1], c_[:, :, 0], c_[:, :, 1]], axis=2)
    wt8 = np.ascontiguousarray(w8.reshape(P, -1).astype(ml_dtypes.float8_e4m3))
    wt16 = np.ascontiguousarray(w16.reshape(P, -1).astype(ml_dtypes.bfloat16))
    return wt16, wt8


def _prep_inputs(x, h, w_ih, w_hh, bsum):
    """Build per-core input maps (host-side cast + transpose)."""
    bf16 = ml_dtypes.bfloat16
    has_bias = bool(np.any(bsum))
    wt16, wt8 = _prep_weights(w_ih, w_hh)

    xb = x.astype(bf16).view(np.uint16)
    hb = h.astype(bf16).view(np.uint16)
    # xh[core, bt*128+f, s*2048 + t*128 + b] = (x if s==0 else h)[.., bt*128+b, t*128+f]
    xc = xb.reshape(N_CORES, NBT, P, NT, P)  # [c, bt, b, t, f]
    hc = hb.reshape(N_CORES, NBT, P, NT, P)
    xh = np.stack([xc, hc], axis=0)  # [s, c, bt, b, t, f]
    xh = np.ascontiguousarray(xh.transpose(1, 2, 5, 0, 4, 3))  # [c, bt, f, s, t, b]
    xh = xh.reshape(N_CORES, NBT * P, 2 * NT * P)
    xh8 = _bf16_to_fp8_lut()[xh]
    xh = xh.view(bf16)
    hn = hb.reshape(N_CORES, B_LOC, D).view(bf16)

    in_maps = []
    for c in range(N_CORES):
        m = {
            "xh": np.ascontiguousarray(xh[c]),
            "xh8": np.ascontiguousarray(xh8[c]),
            "hn": np.ascontiguousarray(hn[c]),
            "wt16": wt16,
            "wt8": wt8,
        }
        if has_bias:
            brep = np.broadcast_to(
                bsum.reshape(1, NUM_BLOCKS * G3), (P, NUM_BLOCKS * G3)
            ).astype(np.float32)
            m["bias"] = np.ascontiguousarray(brep)
        in_maps.append(m)
    return has_bias, in_maps


def kernel(x, h, w_ih, w_hh, b_ih, b_hh, _reps=1, _nc=None):
    from concourse.bass_utils import run_bass_kernel_spmd

    x = np.asarray(x, dtype=np.float32)
    h = np.asarray(h, dtype=np.float32)
    w_ih = np.asarray(w_ih, dtype=np.float32)
    w_hh = np.asarray(w_hh, dtype=np.float32)
    bsum = np.asarray(b_ih, dtype=np.float32) + np.asarray(b_hh, dtype=np.float32)

    has_bias, in_maps = _prep_inputs(x, h, w_ih, w_hh, bsum)
    nc = _nc if _nc is not None else _get_nc(has_bias, _reps)

    res = run_bass_kernel_spmd(nc, in_maps, core_ids=list(range(N_CORES)))
    out = np.concatenate([res.results[c]["out"] for c in range(N_CORES)], axis=0)
    return np.ascontiguousarray(out.astype(np.float32))


# revision 18
# speedup vs baseline: 1.0414x; 1.0034x over previous
"""BlockDiagonalGRU Trainium2 kernel.

Math (per batch row b, per block n of 8, BLK=256):
  gates[b, n, :] = x[b, n*256:(n+1)*256] @ w_ih[n].T + h[b, ...] @ w_hh[n].T + b_ih[n] + b_hh[n]
  r = sigmoid(gates[..., 0:256]); u = sigmoid(gates[..., 256:512])
  c = tanh(r * gates[..., 512:768])
  h_new = (1-u)*h_blk + u*c

Sharding: data-parallel over batch across 8 cores (2048 rows each), weights
replicated (pre-transposed/cast on host).

All transposes are done on the host: per batch tile of 128 rows, the DRAM
tensor `xh` holds the feature-major layout [128 f_lo, s(2), 16 f_hi, 128 b]
in bf16 so matmul lhsT chunks are direct SBUF slices (no PE transposes).
Mixed precision on the PE (validated against the reference in fp64 numpy,
rel err ~1.5e-2 vs the 2e-2 gate):
  - r gate: fp8 e4m3 both paths (DoubleRow perf mode, 2x PE rate); its
    error is damped through tanh(r*g_c)
  - u gate: fp8 x-path, bf16 h-path
  - c gate: bf16 both paths
Inputs are duplicated in fp8 (scaled by ALPHA=1/4, fp8 weights by 4) so
PSUM lands at true scale. h is additionally loaded natural [b, f] bf16 for
the blend. Per core, software-pipelined over 16 batch tiles:
  - HWDGE load xh (1 MB) + xh8 (0.5 MB) two tiles ahead, h_nat (0.5 MB)
    one tile ahead (it is first needed a tile later, easing the startup
    bandwidth crunch); weights loaded block-major in first-use order
  - PE per block: r: 2 fp8 DR matmuls; u: 1 fp8 DR + 2 bf16; c: 4 bf16
  - ACT: one sigmoid over [r|u] PSUM -> SBUF bf16; DVE: r*g_c; ACT: tanh
  - blend in bf16 on DVE, pipelined one tile behind; last two tiles blend
    per-group to shorten the tail; HWDGE store bf16 (host casts to fp32).
"""

import numpy as np
import ml_dtypes

NUM_BLOCKS = 8
BLK = 256
D = 2048
B = 16384
N_CORES = 8
B_LOC = B // N_CORES  # 2048
P = 128
NBT = B_LOC // P  # 16 batch tiles per core
KC = 2  # k-chunks of 128 per block (256 feat)
G3 = 3 * BLK  # 768
BPG = 2  # blocks per PSUM group
NGRP = NUM_BLOCKS // BPG  # 4
NT = D // P  # 16 feat chunks
ALPHA = 0.25  # fp8 scale split: x*ALPHA, w8/ALPHA

_nc_cache = {}
_lut_cache = {}


def _build(has_bias, reps=1):
    import concourse.mybir as mybir
    import concourse.tile as tile
    from concourse import bacc
    from concourse.masks import make_identity

    f32 = mybir.dt.float32
    bf16 = mybir.dt.bfloat16
    fp8 = mybir.dt.float8e4
    DR = mybir.MatmulPerfMode.DoubleRow
    Sig = mybir.ActivationFunctionType.Sigmoid
    Tanh = mybir.ActivationFunctionType.Tanh

    nc = bacc.Bacc(None, target_bir_lowering=False)

    xh_d = nc.dram_tensor("xh", [NBT * P, 2 * NT * P], bf16, kind="ExternalInput")
    xh8_d = nc.dram_tensor("xh8", [NBT * P, 2 * NT * P], fp8, kind="ExternalInput")
    hn_d = nc.dram_tensor("hn", [B_LOC, D], bf16, kind="ExternalInput")
    # bf16 weights, block-major: [p, n, {u_s1, c_s0, c_s1}(3), kc, BLK]
    wt16_d = nc.dram_tensor("wt16", [P, NUM_BLOCKS * 3 * KC * BLK], bf16, kind="ExternalInput")
    # fp8 weights, block-major: [p, n, {r_s0, r_s1, u_s0}(3), kc, BLK]
    wt8_d = nc.dram_tensor("wt8", [P, NUM_BLOCKS * 3 * KC * BLK], fp8, kind="ExternalInput")
    if has_bias:
        bias_d = nc.dram_tensor("bias", [P, NUM_BLOCKS * G3], f32, kind="ExternalInput")
    out_d = nc.dram_tensor("out", [B_LOC, D], bf16, kind="ExternalOutput")
    warm_d = nc.dram_tensor("warm_scratch", [P, P], mybir.dt.bfloat16)

    with tile.TileContext(nc) as tc:
        with (
            tc.tile_pool(name="const", bufs=1) as cpool,
            tc.tile_pool(name="io", bufs=4) as io,
            tc.tile_pool(name="work", bufs=2) as work,
            tc.tile_pool(name="psru", bufs=3, space="PSUM") as psru_pool,
            tc.tile_pool(name="psc", bufs=2, space="PSUM") as psc_pool,
        ):
            ident = cpool.tile([P, P], bf16)
            make_identity(nc, ident)
            wt16 = cpool.tile([P, NUM_BLOCKS, 3, KC, BLK], bf16)
            wt8 = cpool.tile([P, NUM_BLOCKS, 3, KC, BLK], fp8)
            if has_bias:
                bias_sb = cpool.tile([P, NUM_BLOCKS, 3, BLK], f32)

            def load_wt_block(n):
                # one block's fp8 and bf16 weights; alternate HWDGE rings so
                # the chunks share the SDMA engines during the startup crunch
                cw = 3 * KC * BLK
                eng8 = nc.scalar if n % 2 == 0 else nc.sync
                eng16 = nc.sync if n % 2 == 0 else nc.scalar
                eng8.dma_start(wt8[:, n, :, :, :], wt8_d[:, n * cw : (n + 1) * cw])
                eng16.dma_start(wt16[:, n, :, :, :], wt16_d[:, n * cw : (n + 1) * cw])

            def load_tile(bt):
                row0 = bt * P
                xh_t = io.tile([P, 2 * NT * P], bf16, tag="xh", name="xh")
                nc.gpsimd.dma_start(xh_t[:], xh_d[row0 : row0 + P, :])
                xh8_t = io.tile([P, 2 * NT * P], fp8, tag="xh8", name="xh8")
                nc.gpsimd.dma_start(xh8_t[:], xh8_d[row0 : row0 + P, :])
                return xh_t, xh8_t

            def load_hn(bt):
                row0 = bt * P
                hn_t = io.tile([P, D], bf16, tag="hn", name="hn")
                nc.gpsimd.dma_start(hn_t[:], hn_d[row0 : row0 + P, :])
                return hn_t

            def gates_group(bt, grp, xh_t, xh8_t, u_buf, c_buf, fine=None):
                ps_ru = psru_pool.tile([P, BPG * 2 * BLK], f32, tag="psru", name="psru")
                ps_c = psc_pool.tile([P, BPG * BLK], f32, tag="psc", name="psc")
                def lhsT8(s, n):
                    c0 = (s * NT + n * KC) * P
                    return xh8_t[:, c0 : c0 + 2 * P].rearrange("p (k b) -> p k b", k=2)

                # finish the whole [r|u] PSUM tile first so the sigmoid can
                # overlap the c-gate matmuls (and the post-last-matmul tail
                # chain does not wait on the sigmoid)
                for nn in range(BPG):
                    n = grp * BPG + nn
                    ru0 = nn * 2 * BLK
                    # r gate: fp8 DoubleRow, both paths
                    for s in (0, 1):
                        nc.tensor.matmul(
                            ps_ru[:, ru0 : ru0 + BLK],
                            lhsT8(s, n),
                            wt8[:, n, s, :, :],
                            start=(s == 0),
                            stop=(s == 1),
                            perf_mode=DR,
                        )
                    # u gate: fp8 DR x-path, then bf16 h-path
                    nc.tensor.matmul(
                        ps_ru[:, ru0 + BLK : ru0 + 2 * BLK],
                        lhsT8(0, n),
                        wt8[:, n, 2, :, :],
                        start=True,
                        stop=False,
                        perf_mode=DR,
                    )
                    for kc in range(KC):
                        lhsT = xh_t[:, (NT + n * KC + kc) * P :][:, :P]
                        nc.tensor.matmul(
                            ps_ru[:, ru0 + BLK : ru0 + 2 * BLK],
                            lhsT,
                            wt16[:, n, 0, kc, :],
                            start=False,
                            stop=(kc == KC - 1),
                        )
                for nn in range(BPG):
                    n = grp * BPG + nn
                    # c gate: bf16 both paths
                    for s in (0, 1):
                        for kc in range(KC):
                            lhsT = xh_t[:, (s * NT + n * KC + kc) * P :][:, :P]
                            nc.tensor.matmul(
                                ps_c[:, nn * BLK : (nn + 1) * BLK],
                                lhsT,
                                wt16[:, n, 1 + s, kc, :],
                                start=(s == 0 and kc == 0),
                                stop=(s == 1 and kc == KC - 1),
                            )
                if has_bias:
                    for nn in range(BPG):
                        n = grp * BPG + nn
                        for g in range(2):
                            sl = slice(nn * 2 * BLK + g * BLK, nn * 2 * BLK + (g + 1) * BLK)
                            nc.vector.tensor_add(ps_ru[:, sl], ps_ru[:, sl], bias_sb[:, n, g, :])
                        nc.vector.tensor_add(
                            ps_c[:, nn * BLK : (nn + 1) * BLK],
                            ps_c[:, nn * BLK : (nn + 1) * BLK],
                            bias_sb[:, n, 2, :],
                        )
                if fine is not None:
                    # last-tile drain: per-block activations + blend + store so
                    # the post-matmul tail chain is as short as possible
                    bt_, h_nat = fine
                    row0 = bt_ * P
                    for nn in range(BPG):
                        n = grp * BPG + nn
                        c0 = n * BLK
                        nc.scalar.activation(
                            u_buf[:, n * 2 * BLK : (n + 1) * 2 * BLK],
                            ps_ru[:, nn * 2 * BLK : (nn + 1) * 2 * BLK],
                            Sig,
                        )
                        rcb = work.tile([P, BLK], bf16, tag="rcb", name="rcb", bufs=2)
                        nc.vector.tensor_mul(
                            rcb[:],
                            u_buf[:, n * 2 * BLK : n * 2 * BLK + BLK],
                            ps_c[:, nn * BLK : (nn + 1) * BLK],
                        )
                        nc.scalar.activation(c_buf[:, c0 : c0 + BLK], rcb[:], Tanh)
                        d_b = work.tile([P, BLK], bf16, tag="d_b", name="d_b", bufs=2)
                        e_b = work.tile([P, BLK], bf16, tag="e_b", name="e_b", bufs=2)
                        nc.vector.tensor_sub(
                            d_b[:], c_buf[:, c0 : c0 + BLK], h_nat[:, c0 : c0 + BLK]
                        )
                        nc.vector.tensor_mul(
                            e_b[:], u_buf[:, n * 2 * BLK + BLK : (n + 1) * 2 * BLK], d_b[:]
                        )
                        hnew = work.tile([P, BLK], bf16, tag="hnew_b", name="hnew_b", bufs=2)
                        nc.vector.tensor_add(hnew[:], h_nat[:, c0 : c0 + BLK], e_b[:])
                        nc.sync.dma_start(out_d[row0 : row0 + P, c0 : c0 + BLK], hnew[:])
                    return
                col0 = grp * BPG * BLK
                col1 = (grp + 1) * BPG * BLK
                # one sigmoid over the whole [r|u] PSUM tile -> interleaved ru_buf
                ruc0 = grp * BPG * 2 * BLK
                ruc1 = (grp + 1) * BPG * 2 * BLK
                nc.scalar.activation(u_buf[:, ruc0:ruc1], ps_ru[:], Sig)
                r3 = u_buf[:, ruc0:ruc1].rearrange("p (a g b) -> p a g b", a=BPG, g=2)[
                    :, :, 0, :
                ]
                rc = work.tile([P, BPG * BLK], bf16, tag="rc", name="rc", bufs=3)
                nc.vector.tensor_mul(
                    rc[:].rearrange("p (a b) -> p a b", a=BPG),
                    r3,
                    ps_c[:].rearrange("p (a b) -> p a b", a=BPG),
                )
                nc.scalar.activation(c_buf[:, col0:col1], rc[:], Tanh)

            def u_view(u_buf, col0, col1):
                # u slices of the interleaved [r|u] buffer covering hidden
                # columns [col0, col1)
                nblk = (col1 - col0) // BLK
                return u_buf[:, 2 * col0 : 2 * col1].rearrange(
                    "p (a g b) -> p a g b", a=nblk, g=2
                )[:, :, 1, :]

            def blend_full(bt, h_nat, u_buf, c_buf):
                row0 = bt * P
                d_t = work.tile([P, D], bf16, tag="d_t", name="d_t")
                e_t = work.tile([P, D], bf16, tag="e_t", name="e_t")
                nc.vector.tensor_sub(d_t[:], c_buf[:], h_nat[:])
                nc.vector.tensor_mul(
                    e_t[:].rearrange("p (a b) -> p a b", b=BLK),
                    u_view(u_buf, 0, D),
                    d_t[:].rearrange("p (a b) -> p a b", b=BLK),
                )
                hnew = work.tile([P, D], bf16, tag="hnew", name="hnew")
                nc.vector.tensor_add(hnew[:], h_nat[:], e_t[:])
                nc.sync.dma_start(out_d[row0 : row0 + P, :], hnew[:])

            def blend_grp(bt, grp, h_nat, u_buf, c_buf):
                row0 = bt * P
                col0 = grp * BPG * BLK
                col1 = (grp + 1) * BPG * BLK
                d_t = work.tile([P, BPG * BLK], bf16, tag="d_g", name="d_g")
                e_t = work.tile([P, BPG * BLK], bf16, tag="e_g", name="e_g")
                nc.vector.tensor_sub(d_t[:], c_buf[:, col0:col1], h_nat[:, col0:col1])
                nc.vector.tensor_mul(
                    e_t[:].rearrange("p (a b) -> p a b", b=BLK),
                    u_view(u_buf, col0, col1),
                    d_t[:].rearrange("p (a b) -> p a b", b=BLK),
                )
                hnew = work.tile([P, BPG * BLK], bf16, tag="hnew_g", name="hnew_g")
                nc.vector.tensor_add(hnew[:], h_nat[:, col0:col1], e_t[:])
                nc.sync.dma_start(out_d[row0 : row0 + P, col0:col1], hnew[:])

            def warmup():
                # dummy matmuls while the first loads are in flight: keeps the
                # PE HAM activity monitor busy so real matmuls start at 2.4GHz
                ps = psc_pool.tile([P, BPG * BLK], f32, tag="psc", name="psc_warm")
                NWU = 100
                for i in range(NWU):
                    nc.tensor.matmul(
                        ps[:, 0:P],
                        ident[:],
                        ident[:],
                        start=(i == 0),
                        stop=(i == NWU - 1),
                    )
                sc = work.tile([P, P], bf16, tag="warm_sb", name="warm_sb", bufs=1)
                nc.vector.tensor_copy(sc[:], ps[:, 0:P])
                nc.scalar.dma_start(warm_d[:, :], sc[:])

            def load_tile_half(bt, half, xh_t=None, xh8_t=None):
                # load the chunks covering blocks [half*4, half*4+4) of both
                # sources (x and h) so early groups can start on a half tile
                row0 = bt * P
                if xh_t is None:
                    xh_t = io.tile([P, 2 * NT * P], bf16, tag="xh", name="xh")
                    xh8_t = io.tile([P, 2 * NT * P], fp8, tag="xh8", name="xh8")
                c0 = half * NT * P // 2
                for s in (0, 1):
                    lo = s * NT * P + c0
                    hi = lo + NT * P // 2
                    nc.gpsimd.dma_start(xh_t[:, lo:hi], xh_d[row0 : row0 + P, lo:hi])
                    nc.gpsimd.dma_start(xh8_t[:, lo:hi], xh8_d[row0 : row0 + P, lo:hi])
                return xh_t, xh8_t

            def body(_iv=None):
                warmup()
                # startup in strict first-use order: half-tiles interleaved
                # with weight blocks so the first matmuls are gated on as few
                # bytes as possible
                nats = {0: load_tile_half(0, 0)}
                load_wt_block(0)
                load_wt_block(1)
                load_tile_half(0, 1, *nats[0])
                load_wt_block(2)
                load_wt_block(3)
                nats[1] = load_tile_half(1, 0)
                load_wt_block(4)
                load_wt_block(5)
                load_tile_half(1, 1, *nats[1])
                load_wt_block(6)
                load_wt_block(7)
                hns = {0: load_hn(0)}
                if has_bias:
                    nc.sync.dma_start(bias_sb[:, :, :, :], bias_d[:, :])
                pending = None
                for bt in range(NBT):
                    xh_t, xh8_t = nats.pop(bt)
                    h_nat = hns.pop(bt)

                    # interleaved [r|u] sigmoid outputs: [128, n(8) x {r,u} x 256]
                    u_buf = work.tile([P, 2 * D], bf16, tag="u_buf", name="u_buf", bufs=3)
                    c_buf = work.tile([P, D], bf16, tag="c_buf", name="c_buf", bufs=3)

                    for grp in range(NGRP):
                        if grp == 2 and bt + 1 < NBT:
                            # h natural is first consumed a tile after its
                            # gates, so one-tile prefetch is enough
                            hns[bt + 1] = load_hn(bt + 1)
                        if grp == 3 and bt + 2 < NBT:
                            # prefetch two tiles ahead, late in the loop so the
                            # startup weight DMAs win the early SDMA bandwidth
                            nats[bt + 2] = load_tile(bt + 2)
                        gates_group(bt, grp, xh_t, xh8_t, u_buf, c_buf)
                        if bt >= NBT - 2:
                            blend_grp(bt, grp, h_nat, u_buf, c_buf)
                    if pending is not None:
                        blend_full(*pending)
                        pending = None
                    if bt < NBT - 2:
                        pending = (bt, h_nat, u_buf, c_buf)

            if reps == 1:
                body()
            else:
                with tc.For_i(0, reps, 1) as iv:
                    body(iv)

    nc.compile()
    return nc


def _get_nc(has_bias, reps=1):
    key = (has_bias, reps)
    if key not in _nc_cache:
        _nc_cache[key] = _build(has_bias, reps)
    return _nc_cache[key]


def _bf16_to_fp8_lut():
    # LUT over all bf16 bit patterns: fp8e4m3(ALPHA * value)
    if "lut" not in _lut_cache:
        allbits = np.arange(65536, dtype=np.uint16)
        vals = allbits.view(ml_dtypes.bfloat16).astype(np.float32)
        _lut_cache["lut"] = (vals * ALPHA).astype(ml_dtypes.float8_e4m3)
    return _lut_cache["lut"]


def _prep_weights(w_ih, w_hh):
    # wt8 [p, n, {r_s0, r_s1, u_s0}, kc, :] (scaled 1/ALPHA, fp8)
    # wt16[p, n, {u_s1, c_s0, c_s1}, kc, :] (bf16)
    def tr(w):
        # [n, g3, kc, p] -> [p, n, kc, g3]
        return w.reshape(NUM_BLOCKS, G3, KC, P).transpose(3, 0, 2, 1)

    wb = np.stack([tr(w_ih), tr(w_hh)], axis=2)  # [p, n, s, kc, g3]
    r_ = wb[..., 0:BLK]
    u_ = wb[..., BLK : 2 * BLK]
    c_ = wb[..., 2 * BLK :]
    w8 = np.stack([r_[:, :, 0], r_[:, :, 1], u_[:, :, 0]], axis=2) * (1.0 / ALPHA)
    w16 = np.stack([u_[:, :, 1], c_[:, :, 0], c_[:, :, 1]], axis=2)
    wt8 = np.ascontiguousarray(w8.reshape(P, -1).astype(ml_dtypes.float8_e4m3))
    wt16 = np.ascontiguousarray(w16.reshape(P, -1).astype(ml_dtypes.bfloat16))
    return wt16, wt8


def _prep_inputs(x, h, w_ih, w_hh, bsum):
    """Build per-core input maps (host-side cast + transpose)."""
    bf16 = ml_dtypes.bfloat16
    has_bias = bool(np.any(bsum))
    wt16, wt8 = _prep_weights(w_ih, w_hh)

    xb = x.astype(bf16).view(np.uint16)
    hb = h.astype(bf16).view(np.uint16)
    # xh[core, bt*128+f, s*2048 + t*128 + b] = (x if s==0 else h)[.., bt*128+b, t*128+f]
    xc = xb.reshape(N_CORES, NBT, P, NT, P)  # [c, bt, b, t, f]
    hc = hb.reshape(N_CORES, NBT, P, NT, P)
    xh = np.stack([xc, hc], axis=0)  # [s, c, bt, b, t, f]
    xh = np.ascontiguousarray(xh.transpose(1, 2, 5, 0, 4, 3))  # [c, bt, f, s, t, b]
    xh = xh.reshape(N_CORES, NBT * P, 2 * NT * P)
    xh8 = _bf16_to_fp8_lut()[xh]
    xh = xh.view(bf16)
    hn = hb.reshape(N_CORES, B_LOC, D).view(bf16)

    in_maps = []
    for c in range(N_CORES):
        m = {
            "xh": np.ascontiguousarray(xh[c]),
            "xh8": np.ascontiguousarray(xh8[c]),
            "hn": np.ascontiguousarray(hn[c]),
            "wt16": wt16,
            "wt8": wt8,
        }
        if has_bias:
            brep = np.broadcast_to(
                bsum.reshape(1, NUM_BLOCKS * G3), (P, NUM_BLOCKS * G3)
            ).astype(np.float32)
            m["bias"] = np.ascontiguousarray(brep)
        in_maps.append(m)
    return has_bias, in_maps


def kernel(x, h, w_ih, w_hh, b_ih, b_hh, _reps=1, _nc=None):
    from concourse.bass_utils import run_bass_kernel_spmd

    x = np.asarray(x, dtype=np.float32)
    h = np.asarray(h, dtype=np.float32)
    w_ih = np.asarray(w_ih, dtype=np.float32)
    w_hh = np.asarray(w_hh, dtype=np.float32)
    bsum = np.asarray(b_ih, dtype=np.float32) + np.asarray(b_hh, dtype=np.float32)

    has_bias, in_maps = _prep_inputs(x, h, w_ih, w_hh, bsum)
    nc = _nc if _nc is not None else _get_nc(has_bias, _reps)

    res = run_bass_kernel_spmd(nc, in_maps, core_ids=list(range(N_CORES)))
    out = np.concatenate([res.results[c]["out"] for c in range(N_CORES)], axis=0)
    return np.ascontiguousarray(out.astype(np.float32))


# revision 19
# speedup vs baseline: 1.0732x; 1.0305x over previous
"""BlockDiagonalGRU Trainium2 kernel.

Math (per batch row b, per block n of 8, BLK=256):
  gates[b, n, :] = x[b, n*256:(n+1)*256] @ w_ih[n].T + h[b, ...] @ w_hh[n].T + b_ih[n] + b_hh[n]
  r = sigmoid(gates[..., 0:256]); u = sigmoid(gates[..., 256:512])
  c = tanh(r * gates[..., 512:768])
  h_new = (1-u)*h_blk + u*c

Sharding: data-parallel over batch across 8 cores (2048 rows each), weights
replicated (pre-transposed/cast on host).

All transposes are done on the host: per batch tile of 128 rows, the DRAM
tensor `xh` holds the feature-major layout [128 f_lo, s(2), 16 f_hi, 128 b]
in bf16 so matmul lhsT chunks are direct SBUF slices (no PE transposes).
Mixed precision on the PE (validated against the reference in fp64 numpy,
rel err ~1.5e-2 vs the 2e-2 gate):
  - r gate: fp8 e4m3 both paths (DoubleRow perf mode, 2x PE rate); its
    error is damped through tanh(r*g_c)
  - u gate: fp8 x-path, bf16 h-path
  - c gate: bf16 both paths
Inputs are duplicated in fp8 (scaled by ALPHA=1/4, fp8 weights by 4) so
PSUM lands at true scale. h is additionally loaded natural [b, f] bf16 for
the blend. Per core, software-pipelined over 16 batch tiles:
  - HWDGE load xh (1 MB) + xh8 (0.5 MB) two tiles ahead, h_nat (0.5 MB)
    one tile ahead (it is first needed a tile later, easing the startup
    bandwidth crunch); weights loaded block-major in first-use order
  - PE per block: r: 2 fp8 DR matmuls; u: 1 fp8 DR + 2 bf16; c: 4 bf16
  - ACT: one sigmoid over [r|u] PSUM -> SBUF bf16; DVE: r*g_c; ACT: tanh
  - blend in bf16 on DVE, pipelined one tile behind; last two tiles blend
    per-group to shorten the tail; HWDGE store bf16 (host casts to fp32).
"""

import numpy as np
import ml_dtypes

NUM_BLOCKS = 8
BLK = 256
D = 2048
B = 16384
N_CORES = 8
B_LOC = B // N_CORES  # 2048
P = 128
NBT = B_LOC // P  # 16 batch tiles per core
KC = 2  # k-chunks of 128 per block (256 feat)
G3 = 3 * BLK  # 768
BPG = 2  # blocks per PSUM group
NGRP = NUM_BLOCKS // BPG  # 4
NT = D // P  # 16 feat chunks
ALPHA = 0.25  # fp8 scale split: x*ALPHA, w8/ALPHA

_nc_cache = {}
_lut_cache = {}


def _build(has_bias, reps=1):
    import concourse.mybir as mybir
    import concourse.tile as tile
    from concourse import bacc
    from concourse.masks import make_identity

    f32 = mybir.dt.float32
    bf16 = mybir.dt.bfloat16
    fp8 = mybir.dt.float8e4
    DR = mybir.MatmulPerfMode.DoubleRow
    Sig = mybir.ActivationFunctionType.Sigmoid
    Tanh = mybir.ActivationFunctionType.Tanh

    nc = bacc.Bacc(None, target_bir_lowering=False)

    xh_d = nc.dram_tensor("xh", [NBT * P, 2 * NT * P], bf16, kind="ExternalInput")
    xh8_d = nc.dram_tensor("xh8", [NBT * P, 2 * NT * P], fp8, kind="ExternalInput")
    hn_d = nc.dram_tensor("hn", [B_LOC, D], bf16, kind="ExternalInput")
    # bf16 weights, block-major: [p, n, {u_s1, c_s0, c_s1}(3), kc, BLK]
    wt16_d = nc.dram_tensor("wt16", [P, NUM_BLOCKS * 3 * KC * BLK], bf16, kind="ExternalInput")
    # fp8 weights, block-major: [p, n, {r_s0, r_s1, u_s0}(3), kc, BLK]
    wt8_d = nc.dram_tensor("wt8", [P, NUM_BLOCKS * 3 * KC * BLK], fp8, kind="ExternalInput")
    if has_bias:
        bias_d = nc.dram_tensor("bias", [P, NUM_BLOCKS * G3], f32, kind="ExternalInput")
    out_d = nc.dram_tensor("out", [B_LOC, D], bf16, kind="ExternalOutput")
    warm_d = nc.dram_tensor("warm_scratch", [P, P], mybir.dt.bfloat16)

    with tile.TileContext(nc) as tc:
        with (
            tc.tile_pool(name="const", bufs=1) as cpool,
            tc.tile_pool(name="io", bufs=4) as io,
            tc.tile_pool(name="work", bufs=2) as work,
            tc.tile_pool(name="psru", bufs=3, space="PSUM") as psru_pool,
            tc.tile_pool(name="psc", bufs=2, space="PSUM") as psc_pool,
        ):
            ident = cpool.tile([P, P], bf16)
            make_identity(nc, ident)
            wt16 = cpool.tile([P, NUM_BLOCKS, 3, KC, BLK], bf16)
            wt8 = cpool.tile([P, NUM_BLOCKS, 3, KC, BLK], fp8)
            if has_bias:
                bias_sb = cpool.tile([P, NUM_BLOCKS, 3, BLK], f32)

            def load_wt_block(n):
                # one block's fp8 and bf16 weights; alternate HWDGE rings so
                # the chunks share the SDMA engines during the startup crunch
                cw = 3 * KC * BLK
                eng8 = nc.scalar if n % 2 == 0 else nc.sync
                eng16 = nc.sync if n % 2 == 0 else nc.scalar
                eng8.dma_start(wt8[:, n, :, :, :], wt8_d[:, n * cw : (n + 1) * cw])
                eng16.dma_start(wt16[:, n, :, :, :], wt16_d[:, n * cw : (n + 1) * cw])

            def load_tile(bt):
                row0 = bt * P
                xh_t = io.tile([P, 2 * NT * P], bf16, tag="xh", name="xh")
                nc.gpsimd.dma_start(xh_t[:], xh_d[row0 : row0 + P, :])
                xh8_t = io.tile([P, 2 * NT * P], fp8, tag="xh8", name="xh8")
                nc.gpsimd.dma_start(xh8_t[:], xh8_d[row0 : row0 + P, :])
                return xh_t, xh8_t

            def load_hn(bt):
                row0 = bt * P
                hn_t = io.tile([P, D], bf16, tag="hn", name="hn")
                nc.gpsimd.dma_start(hn_t[:], hn_d[row0 : row0 + P, :])
                return hn_t

            def gates_group(bt, grp, xh_t, xh8_t, u_buf, c_buf, fine=None):
                ps_ru = psru_pool.tile([P, BPG * 2 * BLK], f32, tag="psru", name="psru")
                ps_c = psc_pool.tile([P, BPG * BLK], f32, tag="psc", name="psc")
                def lhsT8(s, n):
                    c0 = (s * NT + n * KC) * P
                    return xh8_t[:, c0 : c0 + 2 * P].rearrange("p (k b) -> p k b", k=2)

                # finish the whole [r|u] PSUM tile first so the sigmoid can
                # overlap the c-gate matmuls (and the post-last-matmul tail
                # chain does not wait on the sigmoid)
                for nn in range(BPG):
                    n = grp * BPG + nn
                    ru0 = nn * 2 * BLK
                    # r gate: fp8 DoubleRow, both paths
                    for s in (0, 1):
                        nc.tensor.matmul(
                            ps_ru[:, ru0 : ru0 + BLK],
                            lhsT8(s, n),
                            wt8[:, n, s, :, :],
                            start=(s == 0),
                            stop=(s == 1),
                            perf_mode=DR,
                        )
                    # u gate: fp8 DR x-path, then bf16 h-path
                    nc.tensor.matmul(
                        ps_ru[:, ru0 + BLK : ru0 + 2 * BLK],
                        lhsT8(0, n),
                        wt8[:, n, 2, :, :],
                        start=True,
                        stop=False,
                        perf_mode=DR,
                    )
                    for kc in range(KC):
                        lhsT = xh_t[:, (NT + n * KC + kc) * P :][:, :P]
                        nc.tensor.matmul(
                            ps_ru[:, ru0 + BLK : ru0 + 2 * BLK],
                            lhsT,
                            wt16[:, n, 0, kc, :],
                            start=False,
                            stop=(kc == KC - 1),
                        )
                for nn in range(BPG):
                    n = grp * BPG + nn
                    # c gate: bf16 both paths
                    for s in (0, 1):
                        for kc in range(KC):
                            lhsT = xh_t[:, (s * NT + n * KC + kc) * P :][:, :P]
                            nc.tensor.matmul(
                                ps_c[:, nn * BLK : (nn + 1) * BLK],
                                lhsT,
                                wt16[:, n, 1 + s, kc, :],
                                start=(s == 0 and kc == 0),
                                stop=(s == 1 and kc == KC - 1),
                            )
                if has_bias:
                    for nn in range(BPG):
                        n = grp * BPG + nn
                        for g in range(2):
                            sl = slice(nn * 2 * BLK + g * BLK, nn * 2 * BLK + (g + 1) * BLK)
                            nc.vector.tensor_add(ps_ru[:, sl], ps_ru[:, sl], bias_sb[:, n, g, :])
                        nc.vector.tensor_add(
                            ps_c[:, nn * BLK : (nn + 1) * BLK],
                            ps_c[:, nn * BLK : (nn + 1) * BLK],
                            bias_sb[:, n, 2, :],
                        )
                if fine is not None:
                    # last-tile drain: per-block activations + blend + store so
                    # the post-matmul tail chain is as short as possible
                    bt_, h_nat = fine
                    row0 = bt_ * P
                    for nn in range(BPG):
                        n = grp * BPG + nn
                        c0 = n * BLK
                        nc.scalar.activation(
                            u_buf[:, n * 2 * BLK : (n + 1) * 2 * BLK],
                            ps_ru[:, nn * 2 * BLK : (nn + 1) * 2 * BLK],
                            Sig,
                        )
                        rcb = work.tile([P, BLK], bf16, tag="rcb", name="rcb", bufs=2)
                        nc.vector.tensor_mul(
                            rcb[:],
                            u_buf[:, n * 2 * BLK : n * 2 * BLK + BLK],
                            ps_c[:, nn * BLK : (nn + 1) * BLK],
                        )
                        nc.scalar.activation(c_buf[:, c0 : c0 + BLK], rcb[:], Tanh)
                        d_b = work.tile([P, BLK], bf16, tag="d_b", name="d_b", bufs=2)
                        e_b = work.tile([P, BLK], bf16, tag="e_b", name="e_b", bufs=2)
                        nc.vector.tensor_sub(
                            d_b[:], c_buf[:, c0 : c0 + BLK], h_nat[:, c0 : c0 + BLK]
                        )
                        nc.vector.tensor_mul(
                            e_b[:], u_buf[:, n * 2 * BLK + BLK : (n + 1) * 2 * BLK], d_b[:]
                        )
                        hnew = work.tile([P, BLK], bf16, tag="hnew_b", name="hnew_b", bufs=2)
                        nc.vector.tensor_add(hnew[:], h_nat[:, c0 : c0 + BLK], e_b[:])
                        nc.sync.dma_start(out_d[row0 : row0 + P, c0 : c0 + BLK], hnew[:])
                    return
                col0 = grp * BPG * BLK
                col1 = (grp + 1) * BPG * BLK
                # one sigmoid over the whole [r|u] PSUM tile -> interleaved ru_buf
                ruc0 = grp * BPG * 2 * BLK
                ruc1 = (grp + 1) * BPG * 2 * BLK
                nc.scalar.activation(u_buf[:, ruc0:ruc1], ps_ru[:], Sig)
                r3 = u_buf[:, ruc0:ruc1].rearrange("p (a g b) -> p a g b", a=BPG, g=2)[
                    :, :, 0, :
                ]
                rc = work.tile([P, BPG * BLK], bf16, tag="rc", name="rc", bufs=3)
                nc.vector.tensor_mul(
                    rc[:].rearrange("p (a b) -> p a b", a=BPG),
                    r3,
                    ps_c[:].rearrange("p (a b) -> p a b", a=BPG),
                )
                nc.scalar.activation(c_buf[:, col0:col1], rc[:], Tanh)

            def u_view(u_buf, col0, col1):
                # u slices of the interleaved [r|u] buffer covering hidden
                # columns [col0, col1)
                nblk = (col1 - col0) // BLK
                return u_buf[:, 2 * col0 : 2 * col1].rearrange(
                    "p (a g b) -> p a g b", a=nblk, g=2
                )[:, :, 1, :]

            def blend_full(bt, h_nat, u_buf, c_buf):
                row0 = bt * P
                d_t = work.tile([P, D], bf16, tag="d_t", name="d_t")
                e_t = work.tile([P, D], bf16, tag="e_t", name="e_t")
                nc.vector.tensor_sub(d_t[:], c_buf[:], h_nat[:])
                nc.vector.tensor_mul(
                    e_t[:].rearrange("p (a b) -> p a b", b=BLK),
                    u_view(u_buf, 0, D),
                    d_t[:].rearrange("p (a b) -> p a b", b=BLK),
                )
                hnew = work.tile([P, D], bf16, tag="hnew", name="hnew")
                nc.vector.tensor_add(hnew[:], h_nat[:], e_t[:])
                nc.sync.dma_start(out_d[row0 : row0 + P, :], hnew[:])

            def blend_grp(bt, grp, h_nat, u_buf, c_buf):
                row0 = bt * P
                col0 = grp * BPG * BLK
                col1 = (grp + 1) * BPG * BLK
                d_t = work.tile([P, BPG * BLK], bf16, tag="d_g", name="d_g")
                e_t = work.tile([P, BPG * BLK], bf16, tag="e_g", name="e_g")
                nc.vector.tensor_sub(d_t[:], c_buf[:, col0:col1], h_nat[:, col0:col1])
                nc.vector.tensor_mul(
                    e_t[:].rearrange("p (a b) -> p a b", b=BLK),
                    u_view(u_buf, col0, col1),
                    d_t[:].rearrange("p (a b) -> p a b", b=BLK),
                )
                hnew = work.tile([P, BPG * BLK], bf16, tag="hnew_g", name="hnew_g")
                nc.vector.tensor_add(hnew[:], h_nat[:, col0:col1], e_t[:])
                nc.sync.dma_start(out_d[row0 : row0 + P, col0:col1], hnew[:])

            def warmup():
                # dummy matmuls while the first loads are in flight: keeps the
                # PE HAM activity monitor busy so real matmuls start at 2.4GHz
                ps = psc_pool.tile([P, BPG * BLK], f32, tag="psc", name="psc_warm")
                NWU = 32
                for i in range(NWU):
                    nc.tensor.matmul(
                        ps[:, 0:P],
                        ident[:],
                        ident[:],
                        start=(i == 0),
                        stop=(i == NWU - 1),
                    )
                sc = work.tile([P, P], bf16, tag="warm_sb", name="warm_sb", bufs=1)
                nc.vector.tensor_copy(sc[:], ps[:, 0:P])
                nc.scalar.dma_start(warm_d[:, :], sc[:])

            def load_tile_half(bt, half, xh_t=None, xh8_t=None):
                # load the chunks covering blocks [half*4, half*4+4) of both
                # sources (x and h) so early groups can start on a half tile
                row0 = bt * P
                if xh_t is None:
                    xh_t = io.tile([P, 2 * NT * P], bf16, tag="xh", name="xh")
                    xh8_t = io.tile([P, 2 * NT * P], fp8, tag="xh8", name="xh8")
                c0 = half * NT * P // 2
                for s in (0, 1):
                    lo = s * NT * P + c0
                    hi = lo + NT * P // 2
                    nc.gpsimd.dma_start(xh_t[:, lo:hi], xh_d[row0 : row0 + P, lo:hi])
                    nc.gpsimd.dma_start(xh8_t[:, lo:hi], xh8_d[row0 : row0 + P, lo:hi])
                return xh_t, xh8_t

            def body(_iv=None):
                warmup()
                # startup in strict first-use order: half-tiles interleaved
                # with weight blocks so the first matmuls are gated on as few
                # bytes as possible
                nats = {0: load_tile_half(0, 0)}
                load_wt_block(0)
                load_wt_block(1)
                load_tile_half(0, 1, *nats[0])
                load_wt_block(2)
                load_wt_block(3)
                nats[1] = load_tile_half(1, 0)
                load_wt_block(4)
                load_wt_block(5)
                load_tile_half(1, 1, *nats[1])
                load_wt_block(6)
                load_wt_block(7)
                hns = {0: load_hn(0)}
                if has_bias:
                    nc.sync.dma_start(bias_sb[:, :, :, :], bias_d[:, :])
                pending = None
                for bt in range(NBT):
                    xh_t, xh8_t = nats.pop(bt)
                    h_nat = hns.pop(bt)

                    # interleaved [r|u] sigmoid outputs: [128, n(8) x {r,u} x 256]
                    u_buf = work.tile([P, 2 * D], bf16, tag="u_buf", name="u_buf", bufs=3)
                    c_buf = work.tile([P, D], bf16, tag="c_buf", name="c_buf", bufs=3)

                    for grp in range(NGRP):
                        if grp == 2 and bt + 1 < NBT:
                            # h natural is first consumed a tile after its
                            # gates, so one-tile prefetch is enough
                            hns[bt + 1] = load_hn(bt + 1)
                        if grp == 3 and bt + 2 < NBT:
                            # prefetch two tiles ahead, late in the loop so the
                            # startup weight DMAs win the early SDMA bandwidth
                            nats[bt + 2] = load_tile(bt + 2)
                        gates_group(bt, grp, xh_t, xh8_t, u_buf, c_buf)
                        if bt >= NBT - 2:
                            blend_grp(bt, grp, h_nat, u_buf, c_buf)
                    if pending is not None:
                        blend_full(*pending)
                        pending = None
                    if bt < NBT - 2:
                        pending = (bt, h_nat, u_buf, c_buf)

            if reps == 1:
                body()
            else:
                with tc.For_i(0, reps, 1) as iv:
                    body(iv)

    nc.compile()
    return nc


def _get_nc(has_bias, reps=1):
    key = (has_bias, reps)
    if key not in _nc_cache:
        _nc_cache[key] = _build(has_bias, reps)
    return _nc_cache[key]


def _bf16_to_fp8_lut():
    # LUT over all bf16 bit patterns: fp8e4m3(ALPHA * value)
    if "lut" not in _lut_cache:
        allbits = np.arange(65536, dtype=np.uint16)
        vals = allbits.view(ml_dtypes.bfloat16).astype(np.float32)
        _lut_cache["lut"] = (vals * ALPHA).astype(ml_dtypes.float8_e4m3)
    return _lut_cache["lut"]


def _prep_weights(w_ih, w_hh):
    # wt8 [p, n, {r_s0, r_s1, u_s0}, kc, :] (scaled 1/ALPHA, fp8)
    # wt16[p, n, {u_s1, c_s0, c_s1}, kc, :] (bf16)
    def tr(w):
        # [n, g3, kc, p] -> [p, n, kc, g3]
        return w.reshape(NUM_BLOCKS, G3, KC, P).transpose(3, 0, 2, 1)

    wb = np.stack([tr(w_ih), tr(w_hh)], axis=2)  # [p, n, s, kc, g3]
    r_ = wb[..., 0:BLK]
    u_ = wb[..., BLK : 2 * BLK]
    c_ = wb[..., 2 * BLK :]
    w8 = np.stack([r_[:, :, 0], r_[:, :, 1], u_[:, :, 0]], axis=2) * (1.0 / ALPHA)
    w16 = np.stack([u_[:, :, 1], c_[:, :, 0], c_[:, :, 1]], axis=2)
    wt8 = np.ascontiguousarray(w8.reshape(P, -1).astype(ml_dtypes.float8_e4m3))
    wt16 = np.ascontiguousarray(w16.reshape(P, -1).astype(ml_dtypes.bfloat16))
    return wt16, wt8


def _prep_inputs(x, h, w_ih, w_hh, bsum):
    """Build per-core input maps (host-side cast + transpose)."""
    bf16 = ml_dtypes.bfloat16
    has_bias = bool(np.any(bsum))
    wt16, wt8 = _prep_weights(w_ih, w_hh)

    xb = x.astype(bf16).view(np.uint16)
    hb = h.astype(bf16).view(np.uint16)
    # xh[core, bt*128+f, s*2048 + t*128 + b] = (x if s==0 else h)[.., bt*128+b, t*128+f]
    xc = xb.reshape(N_CORES, NBT, P, NT, P)  # [c, bt, b, t, f]
    hc = hb.reshape(N_CORES, NBT, P, NT, P)
    xh = np.stack([xc, hc], axis=0)  # [s, c, bt, b, t, f]
    xh = np.ascontiguousarray(xh.transpose(1, 2, 5, 0, 4, 3))  # [c, bt, f, s, t, b]
    xh = xh.reshape(N_CORES, NBT * P, 2 * NT * P)
    xh8 = _bf16_to_fp8_lut()[xh]
    xh = xh.view(bf16)
    hn = hb.reshape(N_CORES, B_LOC, D).view(bf16)

    in_maps = []
    for c in range(N_CORES):
        m = {
            "xh": np.ascontiguousarray(xh[c]),
            "xh8": np.ascontiguousarray(xh8[c]),
            "hn": np.ascontiguousarray(hn[c]),
            "wt16": wt16,
            "wt8": wt8,
        }
        if has_bias:
            brep = np.broadcast_to(
                bsum.reshape(1, NUM_BLOCKS * G3), (P, NUM_BLOCKS * G3)
            ).astype(np.float32)
            m["bias"] = np.ascontiguousarray(brep)
        in_maps.append(m)
    return has_bias, in_maps


def kernel(x, h, w_ih, w_hh, b_ih, b_hh, _reps=1, _nc=None):
    from concourse.bass_utils import run_bass_kernel_spmd

    x = np.asarray(x, dtype=np.float32)
    h = np.asarray(h, dtype=np.float32)
    w_ih = np.asarray(w_ih, dtype=np.float32)
    w_hh = np.asarray(w_hh, dtype=np.float32)
    bsum = np.asarray(b_ih, dtype=np.float32) + np.asarray(b_hh, dtype=np.float32)

    has_bias, in_maps = _prep_inputs(x, h, w_ih, w_hh, bsum)
    nc = _nc if _nc is not None else _get_nc(has_bias, _reps)

    res = run_bass_kernel_spmd(nc, in_maps, core_ids=list(range(N_CORES)))
    out = np.concatenate([res.results[c]["out"] for c in range(N_CORES)], axis=0)
    return np.ascontiguousarray(out.astype(np.float32))
